# revision 20
# baseline (speedup 1.0000x reference)
"""Cross-attention kernel for Trainium2, SPMD across 8 NeuronCores.

Math (reference):
    qn = l2norm(q_init); kn = l2norm(k_init)
    q = qn@Wq + bq; k = kn@Wk + bk; v = kn@Wv + bv
    scores = q @ k.T                       # [1, N]
    scores = (scores - mean) / (std_ddof1 + 1e-8); clip(+-10); softmax
    out = (attn @ v) @ Wm + bm
    return sigmoid(gamma)*q_init + (1-sigmoid(gamma))*out

Algebraic restructuring:
  - scores_n = kn_n . u + const, with u = Wk @ q^T; the const (q.bk) cancels
    in (x - mean)/std, so bk is never needed.
  - u itself = M^T qi^T * (stuff): q = qn@Wq + bq with qn = qi/||qi||;
    scores scale-invariance under standardization lets us drop the 1/||qi||
    on the qi@M term (global scale on scores cancels); the bias part must be
    scaled by ||qi|| to stay consistent:  u' = qi@M + ||qi||*c,
    M = Wq@Wk^T (folded on host), c = bq@Wk^T.
  - attn@v@Wm + bm = (ctx/sum_e)@W2 + b2 with W2 = Wv@Wm, b2 = bv@Wm + bm
    (weight folding on host), ctx = sum_n e_n kn_n, e_n = exp(clip(z_n)).
  So the kernel is: one streaming pass over k (fp8) computing per-row
  (||k_n||^2, k_n . u), a tiny AllReduce for mean/std, a softmax-weight step,
  a PE pass ctx = sum_n w_n k_n (fp8 DoubleRow), y = ctx@W2 (fp8 DoubleRow),
  a second tiny AllReduce for (y_partial, sum_e), and a final mix.

Precision: k, W2, M, per-row weights all fp8e4m3 (errors average out across
65536 rows and are damped by (1-sigmoid(gamma)) in the output mix; measured
rel err ~1e-3 vs the 2e-2 gate). Scale factors (64x on M/W2, 4x on w) keep
fp8 values in the normal range; they cancel in standardization / are folded
into the final scalar.

Sharding: k_init rows split 8 ways (8192 rows/core) and pair-packed on host
([row_A | row_B] per partition) so pass 2 can use fp8 DoubleRow matmuls.
"""

import sys

import numpy as np

_TRN_REPO = "/opt/trn_rl_repo"
if _TRN_REPO not in sys.path:
    sys.path.insert(0, _TRN_REPO)

import ml_dtypes  # noqa: E402

BF16NP = ml_dtypes.bfloat16
FP8NP = ml_dtypes.float8_e4m3

import concourse.bass as bass  # noqa: E402
import concourse.bacc as bacc  # noqa: E402
import concourse.tile as tile  # noqa: E402
from concourse import mybir  # noqa: E402
from concourse.alu_op_type import AluOpType as alu  # noqa: E402

F32 = mybir.dt.float32
BF = mybir.dt.bfloat16
F8 = mybir.dt.float8e4
F32NP = np.float32
AF = mybir.ActivationFunctionType
AX = mybir.AxisListType
DR = mybir.MatmulPerfMode.DoubleRow

N_CORES = 8
DIM = 1024
HALF = 512
P = 128
N_TOTAL = 65536
ROWS_PER_CORE = N_TOTAL // N_CORES  # 8192
T = ROWS_PER_CORE // P  # 64 tiles of 128 rows
NPAIR = T // 2  # 32 pair-tiles [128, 2048]

# scale factors (must match between host packing and device epilogue)
S_MW = 64.0  # scale on M = Wq@Wk^T and W2 = Wv@Wm uploads
S_W = 4.0  # scale on softmax row-weights w = e*rnorm
# ps_y = (S_W * ctx_true) @ (S_MW * W2_true) -> fold 1/(S_W*S_MW) at the end
Y_UNSCALE = 1.0 / (S_W * S_MW)

# Pool (gpsimd) cannot run accumulating ALU ops through the backend, so
# pass-1 work is split two ways: squares on Act, dots on DVE.
# fp8 DoubleRow would halve pass-2 PE time, but the backend ISA check
# rejects DoubleRow LDWEIGHTS with M=1 stationary tiles; plain fp8 matmuls
# are used instead.
USE_DOUBLE_ROW = False


def build_nc():
    """Builds the SPMD Tile kernel; identical program on all 8 cores."""
    nc = bacc.Bacc(
        "TRN2", target_bir_lowering=False, debug=False, num_devices=N_CORES
    )

    kk8 = nc.dram_tensor("kk8", [NPAIR * P, 2 * DIM], F8, kind="ExternalInput").ap()
    qinit = nc.dram_tensor("qinit", [1, DIM], F32, kind="ExternalInput").ap()
    mpack = nc.dram_tensor("mpack", [4 * P, 2 * DIM], F8, kind="ExternalInput").ap()
    w2pack = nc.dram_tensor("w2pack", [4 * P, 2 * DIM], F8, kind="ExternalInput").ap()
    cq = nc.dram_tensor("cq", [1, DIM], F32, kind="ExternalInput").ap()
    b2 = nc.dram_tensor("b2", [1, DIM], F32, kind="ExternalInput").ap()
    gamma = nc.dram_tensor("gamma", [1, 1], F32, kind="ExternalInput").ap()
    out = nc.dram_tensor("out", [1, DIM], F32, kind="ExternalOutput").ap()

    rg = [list(range(N_CORES))]

    with tile.TileContext(nc) as tc:
        with (
            tc.tile_pool(name="consts", bufs=1) as cpool,
            tc.tile_pool(name="smallf", bufs=1) as fpool,
            tc.tile_pool(name="wts", bufs=1) as wpool,
            tc.tile_pool(name="stash", bufs=1) as stpool,
            tc.tile_pool(name="junk", bufs=1) as jpool,
            tc.tile_pool(name="psum", bufs=1, space="PSUM") as ppool,
            tc.tile_pool(name="dram", bufs=1, space="DRAM") as dpool,
        ):
            # ---------------- constants / scratch ----------------
            one_f32 = cpool.tile([1, 1], F32, name="one_f32")
            nc.vector.memset(one_f32, 1.0)
            one_f8 = cpool.tile([1, 1], F8, name="one_f8")
            nc.vector.memset(one_f8, 1.0)
            ones_row_b = cpool.tile([1, P], BF, name="ones_row_b")
            nc.vector.memset(ones_row_b, 1.0)
            ones_row_f = cpool.tile([1, P], F32, name="ones_row_f")
            nc.vector.memset(ones_row_f, 1.0)
            ones_col_f = cpool.tile([P, 1], F32, name="ones_col_f")
            nc.vector.memset(ones_col_f, 1.0)
            b1s = fpool.tile([1, 8], F32, name="b1s")
            nc.vector.memset(b1s, 0.0)
            ystage = fpool.tile([1, 1032], F32, name="ystage")
            nc.vector.memset(ystage[0:1, 1024:1032], 0.0)

            # Act table preloads (Square first: needed by first k tile;
            # Rsqrt next for the q-norm; Exp/Sigmoid slotted a bit later).
            tbl = fpool.tile([1, 1], F32, name="tbl")
            nc.vector.memset(tbl, 1.0)
            tblo = fpool.tile([1, 4], F32, name="tblo")
            nc.scalar.activation(tblo[0:1, 0:1], tbl, AF.Square)
            nc.scalar.activation(tblo[0:1, 1:2], tbl, AF.Sqrt)

            # ---------------- input DMAs (order matters) ----------------
            qi = fpool.tile([1, DIM], F32, name="qi")
            nc.sync.dma_start(qi, qinit)
            gm_sb = fpool.tile([1, 1], F32, name="gm_sb")
            nc.sync.dma_start(gm_sb, gamma)
            cq_sb = fpool.tile([1, DIM], F32, name="cq_sb")
            nc.sync.dma_start(cq_sb, cq)

            # first two k pairs ahead of the M matrix so compute starts early
            stash = []
            for p in range(NPAIR):
                st = stpool.tile([P, 2 * DIM], F8, name=f"st{p}", tag=f"st{p}")
                stash.append(st)

            def dma_pair(p):
                nc.sync.dma_start(stash[p], kk8[p * P : (p + 1) * P, :])

            dma_pair(0)
            dma_pair(1)

            msb = wpool.tile([P, 4 * 2 * DIM], F8, name="msb", tag="msb")
            nc.sync.dma_start(
                msb[:].rearrange("p (c j) -> p c j", c=4),
                mpack.rearrange("(c p) j -> p c j", p=P),
            )

            for p in range(2, NPAIR):
                dma_pair(p)

            # needed only in the epilogue; queued behind the k stream
            w2sb = wpool.tile([P, 4 * 2 * DIM], F8, name="w2sb", tag="w2sb")
            nc.sync.dma_start(
                w2sb[:].rearrange("p (c j) -> p c j", c=4),
                w2pack.rearrange("(c p) j -> p c j", p=P),
            )
            b2_sb = fpool.tile([1, DIM], F32, name="b2_sb")
            nc.sync.dma_start(b2_sb, b2)

            # ---------------- q-side: u' = qi@M + ||qi||*c ----------------
            qjunk = jpool.tile([1, DIM], F32, name="qjunk", tag="qj")
            qs1 = fpool.tile([1, 1], F32, name="qs1")
            nc.vector.scalar_tensor_tensor(
                out=qjunk, in0=qi, scalar=1.0, in1=qi,
                op0=alu.mult, op1=alu.mult, accum_out=qs1,
            )
            qnorm = fpool.tile([1, 1], F32, name="qnorm")
            nc.scalar.activation(qnorm, qs1, AF.Sqrt)  # ||qi||
            cg = fpool.tile([1, DIM], F32, name="cg")
            nc.vector.tensor_scalar_mul(cg, cq_sb, qnorm)

            qi8 = fpool.tile([1, DIM], F8, name="qi8")
            nc.vector.tensor_copy(qi8, qi)
            ps_qT = ppool.tile([P, 8], F32, name="ps_qT", tag="pA")
            for c in range(8):
                nc.tensor.matmul(
                    ps_qT[:, c : c + 1],
                    lhsT=qi8[0:1, c * P : (c + 1) * P],
                    rhs=one_f8[0:1, 0:1],
                    start=True, stop=True,
                )
            qiT8 = fpool.tile([P, 8], F8, name="qiT8")
            nc.vector.tensor_copy(qiT8, ps_qT)

            # u = qi8 @ M (fp8 over 4 chunk-pairs; DoubleRow when enabled)
            ps_u = ppool.tile([1, DIM], F32, name="ps_u", tag="pB")
            for h in range(2):
                for c in range(4):
                    rhs_base = msb[:, c * 2 * DIM : (c + 1) * 2 * DIM]
                    if USE_DOUBLE_ROW:
                        nc.tensor.matmul(
                            ps_u[0:1, h * HALF : (h + 1) * HALF],
                            lhsT=qiT8[:, 2 * c : 2 * c + 2].rearrange(
                                "p (t m) -> p t m", m=1
                            ),
                            rhs=rhs_base.rearrange("p (t x) -> p t x", t=2)[
                                :, :, h * HALF : (h + 1) * HALF
                            ],
                            start=(c == 0), stop=(c == 3),
                            perf_mode=DR,
                        )
                    else:
                        for t in range(2):
                            nc.tensor.matmul(
                                ps_u[0:1, h * HALF : (h + 1) * HALF],
                                lhsT=qiT8[:, 2 * c + t : 2 * c + t + 1],
                                rhs=rhs_base[:, t * DIM + h * HALF : t * DIM + (h + 1) * HALF],
                                start=(c == 0 and t == 0), stop=(c == 3 and t == 1),
                            )
            u_bf = fpool.tile([1, DIM], BF, name="u_bf")
            nc.vector.scalar_tensor_tensor(
                out=u_bf, in0=ps_u[0:1, :], scalar=1.0, in1=cg,
                op0=alu.mult, op1=alu.add,
            )
            # broadcast u across partitions
            ps_ub = ppool.tile([P, DIM], F32, name="ps_ub", tag="pA")
            for h in range(2):
                nc.tensor.matmul(
                    ps_ub[:, h * HALF : (h + 1) * HALF],
                    lhsT=ones_row_b[0:1, :],
                    rhs=u_bf[0:1, h * HALF : (h + 1) * HALF],
                    start=True, stop=True,
                )
            u_rep = fpool.tile([P, DIM], BF, name="u_rep")
            nc.vector.tensor_copy(u_rep, ps_ub)

            # remaining Act tables while the k stream warms up
            nc.scalar.activation(tblo[0:1, 2:3], tbl, AF.Exp)
            nc.scalar.activation(tblo[0:1, 3:4], tbl, AF.Sigmoid)

            # gate + static part of the output mix (on Pool, off critical path)
            g_sb = fpool.tile([1, 1], F32, name="g_sb")
            nc.scalar.activation(g_sb, gm_sb, AF.Sigmoid)
            omg = fpool.tile([1, 1], F32, name="omg")
            nc.vector.tensor_scalar(omg, g_sb, -1.0, 1.0, alu.mult, alu.add)
            tb2 = fpool.tile([1, DIM], F32, name="tb2")
            nc.vector.tensor_scalar_mul(tb2, b2_sb, omg)
            base = fpool.tile([1, DIM], F32, name="base")
            nc.vector.scalar_tensor_tensor(
                out=base, in0=qi, scalar=g_sb, in1=tb2,
                op0=alu.mult, op1=alu.add,
            )

            # ---------------- pass 1: stream k (fp8) ----------------
            ssq = fpool.tile([P, T], F32, name="ssq")
            dotc = fpool.tile([P, T], F32, name="dotc")
            junkA = jpool.tile([P, DIM], F8, name="junkA", tag="jA")
            junkV = jpool.tile([P, DIM], F8, name="junkV", tag="jV")
            junkP = jpool.tile([P, DIM], F8, name="junkP", tag="jP")

            for t in range(T):
                p, half = t // 2, t % 2
                kt = stash[p][:, half * DIM : (half + 1) * DIM]
                nc.scalar.activation(
                    junkA, kt, AF.Square, accum_out=ssq[:, t : t + 1]
                )
                nc.vector.scalar_tensor_tensor(
                    out=junkV, in0=kt, scalar=1.0, in1=u_rep,
                    op0=alu.mult, op1=alu.mult, accum_out=dotc[:, t : t + 1],
                )

            # ---------------- local score stats ----------------
            nrm = fpool.tile([P, T], F32, name="nrm")
            nc.scalar.activation(nrm, ssq, AF.Sqrt)  # ||k_n||
            rn = fpool.tile([P, T], F32, name="rn")
            nc.vector.reciprocal(rn, nrm)  # 1/||k_n||
            s = fpool.tile([P, T], F32, name="s")
            stats2 = fpool.tile([P, 2], F32, name="stats2")
            nc.vector.scalar_tensor_tensor(
                out=s, in0=dotc, scalar=1.0, in1=rn,
                op0=alu.mult, op1=alu.mult, accum_out=stats2[:, 0:1],
            )
            sj = jpool.tile([P, T], BF, name="sj", tag="sj")
            nc.vector.scalar_tensor_tensor(
                out=sj, in0=s, scalar=1.0, in1=s,
                op0=alu.mult, op1=alu.mult, accum_out=stats2[:, 1:2],
            )
            ps_st = ppool.tile([1, 2], F32, name="ps_st", tag="pC")
            nc.tensor.matmul(
                ps_st[0:1, 0:2], lhsT=ones_col_f[:, 0:1], rhs=stats2[:, 0:2],
                start=True, stop=True,
            )
            nc.vector.tensor_copy(b1s[0:1, 0:2], ps_st[0:1, 0:2])

            # ---------------- AllReduce #1: (sum_s, sum_s2) ----------------
            b1in = dpool.tile([1, 8], F32, name="b1in")
            nc.sync.dma_start(b1in, b1s)
            b1out = dpool.tile([1, 8], F32, name="b1out", addr_space="Shared")
            nc.gpsimd.collective_compute(
                "AllReduce", alu.add, replica_groups=rg,
                ins=[b1in.opt()], outs=[b1out.opt()],
            )
            gath1 = fpool.tile([1, 8], F32, name="gath1")
            nc.sync.dma_start(gath1, b1out)

            # a = 1/std, b = -mean*a   (ddof=1; the +1e-8 on std is ~1e-6
            # relative for this data and is dropped)
            ab = fpool.tile([1, 2], F32, name="ab")
            t0 = fpool.tile([1, 1], F32, name="t0")
            nc.vector.scalar_tensor_tensor(
                out=t0, in0=gath1[0:1, 0:1], scalar=1.0 / N_TOTAL,
                in1=gath1[0:1, 0:1], op0=alu.mult, op1=alu.mult,
            )
            var0 = fpool.tile([1, 1], F32, name="var0")
            nc.vector.scalar_tensor_tensor(
                out=var0, in0=t0, scalar=-1.0, in1=gath1[0:1, 1:2],
                op0=alu.mult, op1=alu.add,
            )
            sd = fpool.tile([1, 1], F32, name="sd")
            nc.scalar.activation(sd, var0, AF.Sqrt, scale=1.0 / (N_TOTAL - 1))
            nc.vector.reciprocal(ab[0:1, 0:1], sd)
            nc.vector.scalar_tensor_tensor(
                out=ab[0:1, 1:2], in0=gath1[0:1, 0:1], scalar=-1.0 / N_TOTAL,
                in1=ab[0:1, 0:1], op0=alu.mult, op1=alu.mult,
            )
            ps_ab = ppool.tile([P, 2], F32, name="ps_ab", tag="pC")
            nc.tensor.matmul(
                ps_ab[:, 0:2], lhsT=ones_row_f[0:1, :], rhs=ab[0:1, 0:2],
                start=True, stop=True,
            )
            ab_col = fpool.tile([P, 2], F32, name="ab_col")
            nc.vector.tensor_copy(ab_col, ps_ab)

            # ---------------- softmax weights ----------------
            z = fpool.tile([P, T], F32, name="z")
            nc.vector.tensor_scalar(
                z, s, ab_col[:, 0:1], ab_col[:, 1:2], alu.mult, alu.add
            )
            zc = fpool.tile([P, T], F32, name="zc")
            nc.vector.tensor_scalar(zc, z, 10.0, -10.0, alu.min, alu.max)
            e = fpool.tile([P, T], BF, name="e")
            erow = fpool.tile([P, 1], F32, name="erow")
            nc.scalar.activation(e, zc, AF.Exp, accum_out=erow)
            w8 = fpool.tile([P, T], F8, name="w8")
            nc.vector.scalar_tensor_tensor(
                out=w8, in0=e, scalar=S_W, in1=rn,
                op0=alu.mult, op1=alu.mult,
            )
            ps_se = ppool.tile([1, 1], F32, name="ps_se", tag="pD")
            nc.tensor.matmul(
                ps_se[0:1, 0:1], lhsT=erow[:, 0:1], rhs=ones_col_f[:, 0:1],
                start=True, stop=True,
            )
            nc.scalar.copy(ystage[0:1, 1024:1025], ps_se[0:1, 0:1])

            # ---------------- pass 2: ctx = sum_n w_n k_n (DoubleRow) -------
            ps_ctx = ppool.tile([1, DIM], F32, name="ps_ctx", tag="pB")
            for p in range(NPAIR):
                for h in range(2):
                    if USE_DOUBLE_ROW:
                        nc.tensor.matmul(
                            ps_ctx[0:1, h * HALF : (h + 1) * HALF],
                            lhsT=w8[:, 2 * p : 2 * p + 2].rearrange(
                                "p (t m) -> p t m", m=1
                            ),
                            rhs=stash[p][:].rearrange("p (t x) -> p t x", t=2)[
                                :, :, h * HALF : (h + 1) * HALF
                            ],
                            start=(p == 0), stop=(p == NPAIR - 1),
                            perf_mode=DR,
                        )
                    else:
                        for t in range(2):
                            nc.tensor.matmul(
                                ps_ctx[0:1, h * HALF : (h + 1) * HALF],
                                lhsT=w8[:, 2 * p + t : 2 * p + t + 1],
                                rhs=stash[p][:, t * DIM + h * HALF : t * DIM + (h + 1) * HALF],
                                start=(p == 0 and t == 0),
                                stop=(p == NPAIR - 1 and t == 1),
                            )

            # ---------------- y_part = ctx @ W2 ----------------
            ctx8 = fpool.tile([1, DIM], F8, name="ctx8")
            nc.scalar.copy(ctx8, ps_ctx[0:1, :])
            ps_cT = ppool.tile([P, 8], F32, name="ps_cT", tag="pA")
            for c in range(8):
                nc.tensor.matmul(
                    ps_cT[:, c : c + 1],
                    lhsT=ctx8[0:1, c * P : (c + 1) * P],
                    rhs=one_f8[0:1, 0:1],
                    start=True, stop=True,
                )
            cT8 = fpool.tile([P, 8], F8, name="cT8")
            nc.vector.tensor_copy(cT8, ps_cT)

            ps_y = ppool.tile([1, DIM], F32, name="ps_y", tag="pB")
            for h in range(2):
                for c in range(4):
                    rhs_base = w2sb[:, c * 2 * DIM : (c + 1) * 2 * DIM]
                    if USE_DOUBLE_ROW:
                        nc.tensor.matmul(
                            ps_y[0:1, h * HALF : (h + 1) * HALF],
                            lhsT=cT8[:, 2 * c : 2 * c + 2].rearrange(
                                "p (t m) -> p t m", m=1
                            ),
                            rhs=rhs_base.rearrange("p (t x) -> p t x", t=2)[
                                :, :, h * HALF : (h + 1) * HALF
                            ],
                            start=(c == 0), stop=(c == 3),
                            perf_mode=DR,
                        )
                    else:
                        for t in range(2):
                            nc.tensor.matmul(
                                ps_y[0:1, h * HALF : (h + 1) * HALF],
                                lhsT=cT8[:, 2 * c + t : 2 * c + t + 1],
                                rhs=rhs_base[:, t * DIM + h * HALF : t * DIM + (h + 1) * HALF],
                                start=(c == 0 and t == 0), stop=(c == 3 and t == 1),
                            )
            nc.vector.tensor_copy(ystage[0:1, 0:1024], ps_y[0:1, :])

            # ---------------- AllReduce #2: (y_part, sum_e) ----------------
            b2in = dpool.tile([1, 1032], F32, name="b2in")
            nc.sync.dma_start(b2in, ystage)
            b2out = dpool.tile([1, 1032], F32, name="b2out", addr_space="Shared")
            nc.gpsimd.collective_compute(
                "AllReduce", alu.add, replica_groups=rg,
                ins=[b2in.opt()], outs=[b2out.opt()],
            )
            fin = fpool.tile([1, 1032], F32, name="fin")
            nc.sync.dma_start(fin, b2out)

            # out = base + (omg * Y_UNSCALE / sum_e) * y_sum
            rse = fpool.tile([1, 1], F32, name="rse")
            nc.vector.reciprocal(rse, fin[0:1, 1024:1025])
            scl = fpool.tile([1, 1], F32, name="scl")
            nc.vector.scalar_tensor_tensor(
                out=scl, in0=rse, scalar=Y_UNSCALE, in1=omg,
                op0=alu.mult, op1=alu.mult,
            )
            out_sb = fpool.tile([1, DIM], F32, name="out_sb")
            nc.vector.scalar_tensor_tensor(
                out=out_sb, in0=fin[0:1, 0:1024], scalar=scl, in1=base,
                op0=alu.mult, op1=alu.add,
            )
            nc.sync.dma_start(out, out_sb)

    nc.compile()
    return nc


def _pack_pairs(a, npair):
    """[npair*256, 1024] -> [npair*128, 2048] with row r of pair p =
    [A_r | B_r], A = rows 256p..256p+127, B = rows 256p+128..256p+255."""
    n = a.shape[1]
    return np.ascontiguousarray(
        a.reshape(npair, 2, P, n).transpose(0, 2, 1, 3).reshape(npair * P, 2 * n)
    )


def make_in_maps(inputs):
    """Shard/replicate the full inputs into per-core in_maps."""
    k_init = np.asarray(inputs["k_init"], F32NP)
    q_init = np.asarray(inputs["q_init"], F32NP).reshape(1, DIM)
    Wq = np.asarray(inputs["Wq"], F32NP)
    Wk = np.asarray(inputs["Wk"], F32NP)
    Wv = np.asarray(inputs["Wv"], F32NP)
    Wm = np.asarray(inputs["Wm"], F32NP)
    bq_ = np.asarray(inputs["bq"], F32NP).reshape(1, HALF)
    bv_ = np.asarray(inputs["bv"], F32NP).reshape(1, DIM)
    bm_ = np.asarray(inputs["bm"], F32NP).reshape(1, DIM)
    gamma_ = np.asarray(inputs["gamma"], F32NP).reshape(1, 1)

    # host-side weight folding
    M = (Wq @ Wk.T) * S_MW  # [1024, 1024]
    c_row = (bq_ @ Wk.T)  # [1, 1024]
    W2 = (Wv @ Wm) * S_MW  # [1024, 1024]
    b2_ = bv_ @ Wm + bm_  # [1, 1024]

    mpack = _pack_pairs(M, 4).astype(FP8NP)
    w2pack = _pack_pairs(W2, 4).astype(FP8NP)
    k8 = k_init.astype(FP8NP)

    in_maps = []
    for r in range(N_CORES):
        shard = k8[r * ROWS_PER_CORE : (r + 1) * ROWS_PER_CORE]
        in_maps.append(
            {
                "kk8": _pack_pairs(shard, NPAIR),
                "qinit": q_init,
                "mpack": mpack,
                "w2pack": w2pack,
                "cq": np.ascontiguousarray(c_row),
                "b2": np.ascontiguousarray(b2_),
                "gamma": gamma_,
            }
        )
    return in_maps


_NC_CACHE = {}


def _get_nc():
    if "nc" not in _NC_CACHE:
        _NC_CACHE["nc"] = build_nc()
    return _NC_CACHE["nc"]


def run(inputs, trace: bool = False):
    """Run on hardware; returns (out ndarray [1,1024] f32, BassKernelResults)."""
    from concourse.bass_utils import run_bass_kernel_spmd

    nc = _get_nc()
    in_maps = make_in_maps(inputs)
    res = run_bass_kernel_spmd(
        nc, in_maps, core_ids=list(range(N_CORES)), trace=trace
    )
    out = np.asarray(res.results[0]["out"], F32NP).reshape(1, DIM)
    return out, res


def kernel(**inputs) -> np.ndarray:
    out, _ = run(inputs, trace=False)
    return out


# revision 24
# speedup vs baseline: 1.0080x; 1.0080x over previous
"""Cross-attention kernel for Trainium2, SPMD across 8 NeuronCores.

Math (reference):
    qn = l2norm(q_init); kn = l2norm(k_init)
    q = qn@Wq + bq; k = kn@Wk + bk; v = kn@Wv + bv
    scores = q @ k.T                       # [1, N]
    scores = (scores - mean) / (std_ddof1 + 1e-8); clip(+-10); softmax
    out = (attn @ v) @ Wm + bm
    return sigmoid(gamma)*q_init + (1-sigmoid(gamma))*out

Algebraic restructuring:
  - scores_n = kn_n . u + const, with u = Wk @ q^T; the const (q.bk) cancels
    in (x - mean)/std, so bk is never needed.
  - u itself = M^T qi^T * (stuff): q = qn@Wq + bq with qn = qi/||qi||;
    scores scale-invariance under standardization lets us drop the 1/||qi||
    on the qi@M term (global scale on scores cancels); the bias part must be
    scaled by ||qi|| to stay consistent:  u' = qi@M + ||qi||*c,
    M = Wq@Wk^T (folded on host), c = bq@Wk^T.
  - attn@v@Wm + bm = (ctx/sum_e)@W2 + b2 with W2 = Wv@Wm, b2 = bv@Wm + bm
    (weight folding on host), ctx = sum_n e_n kn_n, e_n = exp(clip(z_n)).
  So the kernel is: one streaming pass over k (fp8) computing per-row
  (||k_n||^2, k_n . u), a tiny AllReduce for mean/std, a softmax-weight step,
  a PE pass ctx = sum_n w_n k_n (fp8 DoubleRow), y = ctx@W2 (fp8 DoubleRow),
  a second tiny AllReduce for (y_partial, sum_e), and a final mix.

Precision: k, W2, M, per-row weights all fp8e4m3 (errors average out across
65536 rows and are damped by (1-sigmoid(gamma)) in the output mix; measured
rel err ~1e-3 vs the 2e-2 gate). Scale factors (64x on M/W2, 4x on w) keep
fp8 values in the normal range; they cancel in standardization / are folded
into the final scalar.

Sharding: k_init rows split 8 ways (8192 rows/core) and pair-packed on host
([row_A | row_B] per partition) so pass 2 can use fp8 DoubleRow matmuls.
"""

import sys

import numpy as np

_TRN_REPO = "/opt/trn_rl_repo"
if _TRN_REPO not in sys.path:
    sys.path.insert(0, _TRN_REPO)

import ml_dtypes  # noqa: E402

BF16NP = ml_dtypes.bfloat16
FP8NP = ml_dtypes.float8_e4m3

import concourse.bass as bass  # noqa: E402
import concourse.bacc as bacc  # noqa: E402
import concourse.tile as tile  # noqa: E402
from concourse import mybir  # noqa: E402
from concourse.alu_op_type import AluOpType as alu  # noqa: E402

F32 = mybir.dt.float32
BF = mybir.dt.bfloat16
F8 = mybir.dt.float8e4
F32NP = np.float32
AF = mybir.ActivationFunctionType
AX = mybir.AxisListType
DR = mybir.MatmulPerfMode.DoubleRow

N_CORES = 8
DIM = 1024
HALF = 512
P = 128
N_TOTAL = 65536
ROWS_PER_CORE = N_TOTAL // N_CORES  # 8192
T = ROWS_PER_CORE // P  # 64 tiles of 128 rows
NPAIR = T // 2  # 32 pair-tiles [128, 2048]

# scale factors (must match between host packing and device epilogue)
S_MW = 64.0  # scale on M = Wq@Wk^T and W2 = Wv@Wm uploads
S_W = 4.0  # scale on softmax row-weights w = e*rnorm
# ps_y = (S_W * ctx_true) @ (S_MW * W2_true) -> fold 1/(S_W*S_MW) at the end
Y_UNSCALE = 1.0 / (S_W * S_MW)

# Pool (gpsimd) cannot run accumulating ALU ops through the backend, so
# pass-1 work is split two ways: squares on Act, dots on DVE.
# fp8 DoubleRow would halve pass-2 PE time, but the backend ISA check
# rejects DoubleRow LDWEIGHTS with M=1 stationary tiles; plain fp8 matmuls
# are used instead.
USE_DOUBLE_ROW = False


def build_nc():
    """Builds the SPMD Tile kernel; identical program on all 8 cores."""
    nc = bacc.Bacc(
        "TRN2", target_bir_lowering=False, debug=False, num_devices=N_CORES
    )

    kk8 = nc.dram_tensor("kk8", [NPAIR * P, 2 * DIM], F8, kind="ExternalInput").ap()
    qinit = nc.dram_tensor("qinit", [1, DIM], F32, kind="ExternalInput").ap()
    mpack = nc.dram_tensor("mpack", [4 * P, 2 * DIM], F8, kind="ExternalInput").ap()
    w2pack = nc.dram_tensor("w2pack", [4 * P, 2 * DIM], F8, kind="ExternalInput").ap()
    cq = nc.dram_tensor("cq", [1, DIM], F32, kind="ExternalInput").ap()
    b2 = nc.dram_tensor("b2", [1, DIM], F32, kind="ExternalInput").ap()
    gamma = nc.dram_tensor("gamma", [1, 1], F32, kind="ExternalInput").ap()
    out = nc.dram_tensor("out", [1, DIM], F32, kind="ExternalOutput").ap()

    rg = [list(range(N_CORES))]

    with tile.TileContext(nc) as tc:
        with (
            tc.tile_pool(name="consts", bufs=1) as cpool,
            tc.tile_pool(name="smallf", bufs=1) as fpool,
            tc.tile_pool(name="wts", bufs=1) as wpool,
            tc.tile_pool(name="stash", bufs=1) as stpool,
            tc.tile_pool(name="junk", bufs=1) as jpool,
            tc.tile_pool(name="psum", bufs=1, space="PSUM") as ppool,
            tc.tile_pool(name="dram", bufs=1, space="DRAM") as dpool,
        ):
            # ---------------- constants / scratch ----------------
            one_f32 = cpool.tile([1, 1], F32, name="one_f32")
            nc.vector.memset(one_f32, 1.0)
            one_f8 = cpool.tile([1, 1], F8, name="one_f8")
            nc.vector.memset(one_f8, 1.0)
            ones_row_b = cpool.tile([1, P], BF, name="ones_row_b")
            nc.vector.memset(ones_row_b, 1.0)
            ones_row_f = cpool.tile([1, P], F32, name="ones_row_f")
            nc.vector.memset(ones_row_f, 1.0)
            ones_col_f = cpool.tile([P, 1], F32, name="ones_col_f")
            nc.vector.memset(ones_col_f, 1.0)
            b1s = fpool.tile([1, 8], F32, name="b1s")
            nc.vector.memset(b1s, 0.0)
            ystage = fpool.tile([1, 1032], F32, name="ystage")
            nc.vector.memset(ystage[0:1, 1024:1032], 0.0)

            # The Act engine caches ~2 activation tables; keep function usage
            # in strict order (Sigmoid, Square, Sqrt, Exp) and preload Sqrt/Exp
            # into idle slots so no load lands on the critical stats tail.
            tbl = fpool.tile([1, 1], F32, name="tbl")
            nc.vector.memset(tbl, 1.0)
            tblo = fpool.tile([1, 4], F32, name="tblo")

            # ---------------- input DMAs (order matters) ----------------
            qi = fpool.tile([1, DIM], F32, name="qi")
            nc.sync.dma_start(qi, qinit)
            gm_sb = fpool.tile([1, 1], F32, name="gm_sb")
            nc.sync.dma_start(gm_sb, gamma)
            cq_sb = fpool.tile([1, DIM], F32, name="cq_sb")
            nc.sync.dma_start(cq_sb, cq)

            # first two k pairs ahead of the M matrix so compute starts early
            stash = []
            for p in range(NPAIR):
                st = stpool.tile([P, 2 * DIM], F8, name=f"st{p}", tag=f"st{p}")
                stash.append(st)

            def dma_pair(p):
                nc.sync.dma_start(stash[p], kk8[p * P : (p + 1) * P, :])

            dma_pair(0)
            dma_pair(1)

            msb = wpool.tile([P, 4 * 2 * DIM], F8, name="msb", tag="msb")
            nc.sync.dma_start(
                msb[:].rearrange("p (c j) -> p c j", c=4),
                mpack.rearrange("(c p) j -> p c j", p=P),
            )

            for p in range(2, NPAIR):
                dma_pair(p)

            # needed only in the epilogue; queued behind the k stream
            w2sb = wpool.tile([P, 4 * 2 * DIM], F8, name="w2sb", tag="w2sb")
            nc.sync.dma_start(
                w2sb[:].rearrange("p (c j) -> p c j", c=4),
                w2pack.rearrange("(c p) j -> p c j", p=P),
            )
            b2_sb = fpool.tile([1, DIM], F32, name="b2_sb")
            nc.sync.dma_start(b2_sb, b2)

            # ---------------- q-side: u' = qi@M + ||qi||*c ----------------
            qjunk = jpool.tile([1, DIM], F32, name="qjunk", tag="qj")
            qs1 = fpool.tile([1, 1], F32, name="qs1")
            nc.vector.scalar_tensor_tensor(
                out=qjunk, in0=qi, scalar=1.0, in1=qi,
                op0=alu.mult, op1=alu.mult, accum_out=qs1,
            )
            qnorm = fpool.tile([1, 1], F32, name="qnorm")
            nc.scalar.activation(qnorm, qs1, AF.Sqrt)  # ||qi||
            cg = fpool.tile([1, DIM], F32, name="cg")
            nc.vector.tensor_scalar_mul(cg, cq_sb, qnorm)

            qi8 = fpool.tile([1, DIM], F8, name="qi8")
            nc.vector.tensor_copy(qi8, qi)
            ps_qT = ppool.tile([P, 8], F32, name="ps_qT", tag="pA")
            for c in range(8):
                nc.tensor.matmul(
                    ps_qT[:, c : c + 1],
                    lhsT=qi8[0:1, c * P : (c + 1) * P],
                    rhs=one_f8[0:1, 0:1],
                    start=True, stop=True,
                )
            qiT8 = fpool.tile([P, 8], F8, name="qiT8")
            nc.vector.tensor_copy(qiT8, ps_qT)

            # u = qi8 @ M (fp8 over 4 chunk-pairs; DoubleRow when enabled)
            ps_u = ppool.tile([1, DIM], F32, name="ps_u", tag="pB")
            for h in range(2):
                for c in range(4):
                    rhs_base = msb[:, c * 2 * DIM : (c + 1) * 2 * DIM]
                    if USE_DOUBLE_ROW:
                        nc.tensor.matmul(
                            ps_u[0:1, h * HALF : (h + 1) * HALF],
                            lhsT=qiT8[:, 2 * c : 2 * c + 2].rearrange(
                                "p (t m) -> p t m", m=1
                            ),
                            rhs=rhs_base.rearrange("p (t x) -> p t x", t=2)[
                                :, :, h * HALF : (h + 1) * HALF
                            ],
                            start=(c == 0), stop=(c == 3),
                            perf_mode=DR,
                        )
                    else:
                        for t in range(2):
                            nc.tensor.matmul(
                                ps_u[0:1, h * HALF : (h + 1) * HALF],
                                lhsT=qiT8[:, 2 * c + t : 2 * c + t + 1],
                                rhs=rhs_base[:, t * DIM + h * HALF : t * DIM + (h + 1) * HALF],
                                start=(c == 0 and t == 0), stop=(c == 3 and t == 1),
                            )
            u_bf = fpool.tile([1, DIM], BF, name="u_bf")
            nc.vector.scalar_tensor_tensor(
                out=u_bf, in0=ps_u[0:1, :], scalar=1.0, in1=cg,
                op0=alu.mult, op1=alu.add,
            )
            # broadcast u across partitions
            ps_ub = ppool.tile([P, DIM], F32, name="ps_ub", tag="pA")
            for h in range(2):
                nc.tensor.matmul(
                    ps_ub[:, h * HALF : (h + 1) * HALF],
                    lhsT=ones_row_b[0:1, :],
                    rhs=u_bf[0:1, h * HALF : (h + 1) * HALF],
                    start=True, stop=True,
                )
            u_rep = fpool.tile([P, DIM], BF, name="u_rep")
            nc.vector.tensor_copy(u_rep, ps_ub)

            # gate + static part of the output mix (off critical path)
            g_sb = fpool.tile([1, 1], F32, name="g_sb")
            nc.scalar.activation(g_sb, gm_sb, AF.Sigmoid)
            omg = fpool.tile([1, 1], F32, name="omg")
            nc.vector.tensor_scalar(omg, g_sb, -1.0, 1.0, alu.mult, alu.add)
            tb2 = fpool.tile([1, DIM], F32, name="tb2")
            nc.vector.tensor_scalar_mul(tb2, b2_sb, omg)
            base = fpool.tile([1, DIM], F32, name="base")
            nc.vector.scalar_tensor_tensor(
                out=base, in0=qi, scalar=g_sb, in1=tb2,
                op0=alu.mult, op1=alu.add,
            )

            # ---------------- pass 1: stream k (fp8) ----------------
            ssq = fpool.tile([P, T], F32, name="ssq")
            dotc = fpool.tile([P, T], F32, name="dotc")
            junkA = jpool.tile([P, DIM], F8, name="junkA", tag="jA")
            junkV = jpool.tile([P, DIM], F8, name="junkV", tag="jV")
            junkP = jpool.tile([P, DIM], F8, name="junkP", tag="jP")

            for t in range(T):
                p, half = t // 2, t % 2
                kt = stash[p][:, half * DIM : (half + 1) * DIM]
                if t == T - 3:
                    # preload the Sqrt table while squares still run
                    nc.scalar.activation(tblo[0:1, 1:2], tbl, AF.Sqrt)
                nc.scalar.activation(
                    junkA, kt, AF.Square, accum_out=ssq[:, t : t + 1]
                )
                nc.vector.scalar_tensor_tensor(
                    out=junkV, in0=kt, scalar=1.0, in1=u_rep,
                    op0=alu.mult, op1=alu.mult, accum_out=dotc[:, t : t + 1],
                )

            # ---------------- local score stats ----------------
            nrm = fpool.tile([P, T], F32, name="nrm")
            nc.scalar.activation(nrm, ssq, AF.Sqrt)  # ||k_n||
            rn = fpool.tile([P, T], F32, name="rn")
            nc.vector.reciprocal(rn, nrm)  # 1/||k_n||
            s = fpool.tile([P, T], F32, name="s")
            stats2 = fpool.tile([P, 2], F32, name="stats2")
            nc.vector.scalar_tensor_tensor(
                out=s, in0=dotc, scalar=1.0, in1=rn,
                op0=alu.mult, op1=alu.mult, accum_out=stats2[:, 0:1],
            )
            sj = jpool.tile([P, T], BF, name="sj", tag="sj")
            nc.vector.scalar_tensor_tensor(
                out=sj, in0=s, scalar=1.0, in1=s,
                op0=alu.mult, op1=alu.mult, accum_out=stats2[:, 1:2],
            )
            # preload the Exp table under the AllReduce wait
            nc.scalar.activation(tblo[0:1, 2:3], tbl, AF.Exp)
            ps_st = ppool.tile([1, 2], F32, name="ps_st", tag="pC")
            nc.tensor.matmul(
                ps_st[0:1, 0:2], lhsT=ones_col_f[:, 0:1], rhs=stats2[:, 0:2],
                start=True, stop=True,
            )
            nc.vector.tensor_copy(b1s[0:1, 0:2], ps_st[0:1, 0:2])

            # ---------------- AllReduce #1: (sum_s, sum_s2) ----------------
            b1in = dpool.tile([1, 8], F32, name="b1in")
            nc.sync.dma_start(b1in, b1s)
            b1out = dpool.tile([1, 8], F32, name="b1out", addr_space="Shared")
            nc.gpsimd.collective_compute(
                "AllReduce", alu.add, replica_groups=rg,
                ins=[b1in.opt()], outs=[b1out.opt()],
            )
            gath1 = fpool.tile([1, 8], F32, name="gath1")
            nc.sync.dma_start(gath1, b1out)

            # a = 1/std, b = -mean*a   (ddof=1; the +1e-8 on std is ~1e-6
            # relative for this data and is dropped)
            ab = fpool.tile([1, 2], F32, name="ab")
            t0 = fpool.tile([1, 1], F32, name="t0")
            nc.vector.scalar_tensor_tensor(
                out=t0, in0=gath1[0:1, 0:1], scalar=1.0 / N_TOTAL,
                in1=gath1[0:1, 0:1], op0=alu.mult, op1=alu.mult,
            )
            var0 = fpool.tile([1, 1], F32, name="var0")
            nc.vector.scalar_tensor_tensor(
                out=var0, in0=t0, scalar=-1.0, in1=gath1[0:1, 1:2],
                op0=alu.mult, op1=alu.add,
            )
            sd = fpool.tile([1, 1], F32, name="sd")
            nc.scalar.activation(sd, var0, AF.Sqrt, scale=1.0 / (N_TOTAL - 1))
            nc.vector.reciprocal(ab[0:1, 0:1], sd)
            nc.vector.scalar_tensor_tensor(
                out=ab[0:1, 1:2], in0=gath1[0:1, 0:1], scalar=-1.0 / N_TOTAL,
                in1=ab[0:1, 0:1], op0=alu.mult, op1=alu.mult,
            )
            ps_ab = ppool.tile([P, 2], F32, name="ps_ab", tag="pC")
            nc.tensor.matmul(
                ps_ab[:, 0:2], lhsT=ones_row_f[0:1, :], rhs=ab[0:1, 0:2],
                start=True, stop=True,
            )
            ab_col = fpool.tile([P, 2], F32, name="ab_col")
            nc.vector.tensor_copy(ab_col, ps_ab)

            # ---------------- softmax weights ----------------
            z = fpool.tile([P, T], F32, name="z")
            nc.vector.tensor_scalar(
                z, s, ab_col[:, 0:1], ab_col[:, 1:2], alu.mult, alu.add
            )
            zc = fpool.tile([P, T], F32, name="zc")
            nc.vector.tensor_scalar(zc, z, 10.0, -10.0, alu.min, alu.max)
            e = fpool.tile([P, T], BF, name="e")
            erow = fpool.tile([P, 1], F32, name="erow")
            nc.scalar.activation(e, zc, AF.Exp, accum_out=erow)
            w8 = fpool.tile([P, T], F8, name="w8")
            nc.vector.scalar_tensor_tensor(
                out=w8, in0=e, scalar=S_W, in1=rn,
                op0=alu.mult, op1=alu.mult,
            )
            ps_se = ppool.tile([1, 1], F32, name="ps_se", tag="pD")
            nc.tensor.matmul(
                ps_se[0:1, 0:1], lhsT=erow[:, 0:1], rhs=ones_col_f[:, 0:1],
                start=True, stop=True,
            )
            nc.scalar.copy(ystage[0:1, 1024:1025], ps_se[0:1, 0:1])

            # ---------------- pass 2: ctx = sum_n w_n k_n (DoubleRow) -------
            ps_ctx = ppool.tile([1, DIM], F32, name="ps_ctx", tag="pB")
            for p in range(NPAIR):
                for h in range(2):
                    if USE_DOUBLE_ROW:
                        nc.tensor.matmul(
                            ps_ctx[0:1, h * HALF : (h + 1) * HALF],
                            lhsT=w8[:, 2 * p : 2 * p + 2].rearrange(
                                "p (t m) -> p t m", m=1
                            ),
                            rhs=stash[p][:].rearrange("p (t x) -> p t x", t=2)[
                                :, :, h * HALF : (h + 1) * HALF
                            ],
                            start=(p == 0), stop=(p == NPAIR - 1),
                            perf_mode=DR,
                        )
                    else:
                        for t in range(2):
                            nc.tensor.matmul(
                                ps_ctx[0:1, h * HALF : (h + 1) * HALF],
                                lhsT=w8[:, 2 * p + t : 2 * p + t + 1],
                                rhs=stash[p][:, t * DIM + h * HALF : t * DIM + (h + 1) * HALF],
                                start=(p == 0 and t == 0),
                                stop=(p == NPAIR - 1 and t == 1),
                            )

            # ---------------- y_part = ctx @ W2 ----------------
            ctx8 = fpool.tile([1, DIM], F8, name="ctx8")
            nc.scalar.copy(ctx8, ps_ctx[0:1, :])
            ps_cT = ppool.tile([P, 8], F32, name="ps_cT", tag="pA")
            for c in range(8):
                nc.tensor.matmul(
                    ps_cT[:, c : c + 1],
                    lhsT=ctx8[0:1, c * P : (c + 1) * P],
                    rhs=one_f8[0:1, 0:1],
                    start=True, stop=True,
                )
            cT8 = fpool.tile([P, 8], F8, name="cT8")
            nc.vector.tensor_copy(cT8, ps_cT)

            ps_y = ppool.tile([1, DIM], F32, name="ps_y", tag="pB")
            for h in range(2):
                for c in range(4):
                    rhs_base = w2sb[:, c * 2 * DIM : (c + 1) * 2 * DIM]
                    if USE_DOUBLE_ROW:
                        nc.tensor.matmul(
                            ps_y[0:1, h * HALF : (h + 1) * HALF],
                            lhsT=cT8[:, 2 * c : 2 * c + 2].rearrange(
                                "p (t m) -> p t m", m=1
                            ),
                            rhs=rhs_base.rearrange("p (t x) -> p t x", t=2)[
                                :, :, h * HALF : (h + 1) * HALF
                            ],
                            start=(c == 0), stop=(c == 3),
                            perf_mode=DR,
                        )
                    else:
                        for t in range(2):
                            nc.tensor.matmul(
                                ps_y[0:1, h * HALF : (h + 1) * HALF],
                                lhsT=cT8[:, 2 * c + t : 2 * c + t + 1],
                                rhs=rhs_base[:, t * DIM + h * HALF : t * DIM + (h + 1) * HALF],
                                start=(c == 0 and t == 0), stop=(c == 3 and t == 1),
                            )
            nc.vector.tensor_copy(ystage[0:1, 0:1024], ps_y[0:1, :])

            # ---------------- AllReduce #2: (y_part, sum_e) ----------------
            b2in = dpool.tile([1, 1032], F32, name="b2in")
            nc.sync.dma_start(b2in, ystage)
            b2out = dpool.tile([1, 1032], F32, name="b2out", addr_space="Shared")
            nc.gpsimd.collective_compute(
                "AllReduce", alu.add, replica_groups=rg,
                ins=[b2in.opt()], outs=[b2out.opt()],
            )
            fin = fpool.tile([1, 1032], F32, name="fin")
            nc.sync.dma_start(fin, b2out)

            # out = base + (omg * Y_UNSCALE / sum_e) * y_sum
            rse = fpool.tile([1, 1], F32, name="rse")
            nc.vector.reciprocal(rse, fin[0:1, 1024:1025])
            scl = fpool.tile([1, 1], F32, name="scl")
            nc.vector.scalar_tensor_tensor(
                out=scl, in0=rse, scalar=Y_UNSCALE, in1=omg,
                op0=alu.mult, op1=alu.mult,
            )
            out_sb = fpool.tile([1, DIM], F32, name="out_sb")
            nc.vector.scalar_tensor_tensor(
                out=out_sb, in0=fin[0:1, 0:1024], scalar=scl, in1=base,
                op0=alu.mult, op1=alu.add,
            )
            nc.sync.dma_start(out, out_sb)

    nc.compile()
    return nc


def _pack_pairs(a, npair):
    """[npair*256, 1024] -> [npair*128, 2048] with row r of pair p =
    [A_r | B_r], A = rows 256p..256p+127, B = rows 256p+128..256p+255."""
    n = a.shape[1]
    return np.ascontiguousarray(
        a.reshape(npair, 2, P, n).transpose(0, 2, 1, 3).reshape(npair * P, 2 * n)
    )


def make_in_maps(inputs):
    """Shard/replicate the full inputs into per-core in_maps."""
    k_init = np.asarray(inputs["k_init"], F32NP)
    q_init = np.asarray(inputs["q_init"], F32NP).reshape(1, DIM)
    Wq = np.asarray(inputs["Wq"], F32NP)
    Wk = np.asarray(inputs["Wk"], F32NP)
    Wv = np.asarray(inputs["Wv"], F32NP)
    Wm = np.asarray(inputs["Wm"], F32NP)
    bq_ = np.asarray(inputs["bq"], F32NP).reshape(1, HALF)
    bv_ = np.asarray(inputs["bv"], F32NP).reshape(1, DIM)
    bm_ = np.asarray(inputs["bm"], F32NP).reshape(1, DIM)
    gamma_ = np.asarray(inputs["gamma"], F32NP).reshape(1, 1)

    # host-side weight folding
    M = (Wq @ Wk.T) * S_MW  # [1024, 1024]
    c_row = (bq_ @ Wk.T)  # [1, 1024]
    W2 = (Wv @ Wm) * S_MW  # [1024, 1024]
    b2_ = bv_ @ Wm + bm_  # [1, 1024]

    mpack = _pack_pairs(M, 4).astype(FP8NP)
    w2pack = _pack_pairs(W2, 4).astype(FP8NP)
    k8 = k_init.astype(FP8NP)

    in_maps = []
    for r in range(N_CORES):
        shard = k8[r * ROWS_PER_CORE : (r + 1) * ROWS_PER_CORE]
        in_maps.append(
            {
                "kk8": _pack_pairs(shard, NPAIR),
                "qinit": q_init,
                "mpack": mpack,
                "w2pack": w2pack,
                "cq": np.ascontiguousarray(c_row),
                "b2": np.ascontiguousarray(b2_),
                "gamma": gamma_,
            }
        )
    return in_maps


_NC_CACHE = {}


def _get_nc():
    if "nc" not in _NC_CACHE:
        _NC_CACHE["nc"] = build_nc()
    return _NC_CACHE["nc"]


def run(inputs, trace: bool = False):
    """Run on hardware; returns (out ndarray [1,1024] f32, BassKernelResults)."""
    from concourse.bass_utils import run_bass_kernel_spmd

    nc = _get_nc()
    in_maps = make_in_maps(inputs)
    res = run_bass_kernel_spmd(
        nc, in_maps, core_ids=list(range(N_CORES)), trace=trace
    )
    out = np.asarray(res.results[0]["out"], F32NP).reshape(1, DIM)
    return out, res


def kernel(**inputs) -> np.ndarray:
    out, _ = run(inputs, trace=False)
    return out
